# revision 6
# baseline (speedup 1.0000x reference)
"""Cross-attention kernel for TRN2, 8 NeuronCores, data-parallel over points.

Math (derived from the reference):
  qp[n]  = Wq @ q[n]                      (needed for the residual)
  scores[n,w] = (q[n] @ (Wq.T Wk) * s) . k[w,n]
  attn[n] = softmax_w(scores[n])          (identical for every query view)
  vmix[n] = sum_w attn[n,w] * v[w,n]      (mix RAW v, then project once)
  y[n]    = gelu(vmix[n] @ (Wo Wv).T + bo) + qp[n]
  out[c][8*i + j] = y[c*4096 + i]         (row replicated 8x, per view slab c)

Per core: 4096 points = 32 tiles of 128 partition-points, grouped by 4
(group-level softmax/epilogue to amortize ACT table loads and small-op
overhead).  vmix scaling runs on the otherwise-idle GpSimd engine.
"""

import numpy as np

import concourse.bass as bass
import concourse.mybir as mybir
import concourse.tile as tile
from concourse import bacc
from concourse.bass_utils import run_bass_kernel_spmd

N_CORES = 8
N_TOTAL = 32768
NC_PTS = N_TOTAL // N_CORES  # 4096 points per core
D = 256
V = 8
P = 128
G = 4  # tiles per group
N_TILES = NC_PTS // P  # 32
F32 = mybir.dt.float32
AX = mybir.AxisListType
OP = mybir.AluOpType
AF = mybir.ActivationFunctionType


def _bcast(ap, axis_count, after_dims):
    """Insert a [0, axis_count] broadcast dim before the last `after_dims`
    dims of `ap`'s access pattern."""
    dims = list(ap.ap)
    pos = len(dims) - after_dims
    dims = dims[:pos] + [[0, axis_count]] + dims[pos:]
    return bass.AP(tensor=ap.tensor, offset=ap.offset, ap=dims)


def build_bass(n_tiles: int = N_TILES, gelu: bool = True):
    nc = bacc.Bacc(
        "TRN2", target_bir_lowering=False, debug=False, num_devices=N_CORES
    )
    assert n_tiles % G == 0
    n_groups = n_tiles // G
    npts = n_tiles * P
    q_d = nc.dram_tensor("q", [npts, D], F32, kind="ExternalInput")
    k_d = nc.dram_tensor("k", [V, npts, D], F32, kind="ExternalInput")
    v_d = nc.dram_tensor("v", [V, npts, D], F32, kind="ExternalInput")
    # mamq: [din_half, MA | MQ] (concatenated moving operands, one MM each)
    mamq_d = nc.dram_tensor("mamq", [D, 2 * D], F32, kind="ExternalInput")
    mb_d = nc.dram_tensor("mb", [D, D], F32, kind="ExternalInput")
    bo_d = nc.dram_tensor("bo_b", [P, D], F32, kind="ExternalInput")
    id_d = nc.dram_tensor("ident", [P, P], F32, kind="ExternalInput")
    out_d = nc.dram_tensor("out", [npts * V, D], F32, kind="ExternalOutput")

    with tile.TileContext(nc) as tc:
        with (
            tc.tile_pool(name="singles", bufs=1) as singles,
            tc.tile_pool(name="io", bufs=3) as io,
            tc.tile_pool(name="gio", bufs=2) as gio,
            tc.tile_pool(name="work", bufs=2) as work,
            tc.tile_pool(name="gwork", bufs=2) as gwork,
            tc.tile_pool(name="pst", bufs=2, space="PSUM") as pst,
            tc.tile_pool(name="ps", bufs=2, space="PSUM") as ps,
        ):
            mamq_t = singles.tile([P, 2, 2 * D], F32)
            mb_t = singles.tile([P, 2, D], F32)
            bo_t = singles.tile([P, D], F32)
            id_t = singles.tile([P, P], F32)
            nc.sync.dma_start(
                out=mamq_t, in_=mamq_d.ap().rearrange("(h p) d -> p h d", p=P)
            )
            nc.sync.dma_start(
                out=mb_t, in_=mb_d.ap().rearrange("(h p) d -> p h d", p=P)
            )
            nc.sync.dma_start(out=bo_t, in_=bo_d.ap())
            nc.sync.dma_start(out=id_t, in_=id_d.ap())

            for gi in range(n_groups):
                g0 = gi * G  # first tile of group
                # group q load: [P, G, D]
                q_g = gio.tile([P, G, D], F32, tag="q")
                nc.sync.dma_start(
                    out=q_g,
                    in_=q_d.ap()[g0 * P : (g0 + G) * P].rearrange(
                        "(g p) d -> p g d", p=P
                    ),
                )
                scores_g = gwork.tile([P, G, V], F32, tag="scores")
                qp_g = gwork.tile([P, G, D], F32, tag="qp")
                y_ps = ps.tile([P, G, D], F32, tag="y")

                k_ts, v_ts = [], []
                for t in range(G):
                    sl = slice((g0 + t) * P, (g0 + t + 1) * P)
                    k_t = io.tile([P, V, D], F32, tag="k")
                    v_t = io.tile([P, V, D], F32, tag="v")
                    nc.sync.dma_start(
                        out=k_t, in_=k_d.ap()[:, sl].rearrange("w p d -> p w d")
                    )
                    nc.sync.dma_start(
                        out=v_t, in_=v_d.ap()[:, sl].rearrange("w p d -> p w d")
                    )
                    k_ts.append(k_t)
                    v_ts.append(v_t)

                for t in range(G):
                    k_t = k_ts[t]
                    # q tile -> [din, n] halves via PE transpose
                    qT_ps = pst.tile([P, 2, P], F32, tag="tps")
                    nc.tensor.transpose(qT_ps[:, 0], q_g[:, t, 0:P], id_t)
                    nc.tensor.transpose(qT_ps[:, 1], q_g[:, t, P:D], id_t)
                    qT_t = work.tile([P, 2, P], F32, tag="qT")
                    nc.vector.tensor_copy(qT_t, qT_ps)

                    # [qk | qp] = q @ [MA | MQ]   (one N=512 MM per K-half)
                    qkqp_ps = ps.tile([P, 2 * D], F32, tag="qkqp")
                    nc.tensor.matmul(
                        qkqp_ps, qT_t[:, 0], mamq_t[:, 0], start=True, stop=False
                    )
                    nc.tensor.matmul(
                        qkqp_ps, qT_t[:, 1], mamq_t[:, 1], start=False, stop=True
                    )
                    qk_ps = qkqp_ps[:, 0:D]
                    # stash qp for the group epilogue (frees the PSUM bank)
                    nc.vector.tensor_copy(qp_g[:, t], qkqp_ps[:, D : 2 * D])

                    # scores: one broadcast mul + one reduce over d
                    scr = work.tile([P, V, D], F32, tag="scr")
                    nc.vector.tensor_tensor(
                        scr, _bcast(qk_ps, V, 1), k_t, op=OP.mult
                    )
                    nc.vector.tensor_reduce(
                        scores_g[:, t], scr, axis=AX.X, op=OP.add
                    )

                # group softmax (unnormalized exp; 1/sum folded into scales)
                mx = gwork.tile([P, G], F32, tag="mx")
                nc.vector.tensor_reduce(mx, scores_g, axis=AX.X, op=OP.max)
                e_g = gwork.tile([P, G, V], F32, tag="e")
                nc.vector.tensor_tensor(
                    e_g, scores_g, _bcast(mx, V, 0), op=OP.subtract
                )
                nc.scalar.activation(e_g, e_g, AF.Exp)
                sm = gwork.tile([P, G], F32, tag="sm")
                nc.vector.tensor_reduce(sm, e_g, axis=AX.X, op=OP.add)
                rs = gwork.tile([P, G], F32, tag="rs")
                nc.vector.reciprocal(rs, sm)

                for t in range(G):
                    v_t = v_ts[t]
                    # scaled views on GpSimd: sv[:,w,:] = v*attn_w*recip
                    sv = work.tile([P, V, D], F32, tag="sv")
                    for w in range(V):
                        nc.gpsimd.tensor_scalar(
                            out=sv[:, w],
                            in0=v_t[:, w],
                            scalar1=e_g[:, t, w : w + 1],
                            scalar2=rs[:, t : t + 1],
                            op0=OP.mult,
                            op1=OP.mult,
                        )
                    # vmix = sum_w sv  (reduce over w via transposed AP)
                    vmix_t = work.tile([P, D], F32, tag="vmix")
                    nc.vector.tensor_reduce(
                        vmix_t, sv.rearrange("p w d -> p d w"), axis=AX.X, op=OP.add
                    )

                    # vmix -> [din, n] halves
                    vT_ps = pst.tile([P, 2, P], F32, tag="tps")
                    nc.tensor.transpose(vT_ps[:, 0], vmix_t[:, 0:P], id_t)
                    nc.tensor.transpose(vT_ps[:, 1], vmix_t[:, P:D], id_t)
                    vT_t = work.tile([P, 2, P], F32, tag="vT")
                    nc.vector.tensor_copy(vT_t, vT_ps)

                    # ylin = vmix @ (Wo Wv).T  -> group PSUM tile
                    nc.tensor.matmul(
                        y_ps[:, t], vT_t[:, 0], mb_t[:, 0], start=True, stop=False
                    )
                    nc.tensor.matmul(
                        y_ps[:, t], vT_t[:, 1], mb_t[:, 1], start=False, stop=True
                    )

                # group epilogue: y = gelu(ylin + bo) + qp
                yb = gwork.tile([P, G, D], F32, tag="yb")
                nc.vector.tensor_tensor(yb, y_ps, _bcast(bo_t, G, 1), op=OP.add)
                gl = gwork.tile([P, G, D], F32, tag="gl")
                nc.scalar.activation(gl, yb, AF.Gelu if gelu else AF.Identity)
                y_out = gio.tile([P, G, D], F32, tag="yout")
                nc.vector.tensor_tensor(y_out, gl, qp_g, op=OP.add)

                # store: each point row replicated 8x (per tile; 4D APs
                # don't balance in the DMA lowering)
                for t in range(G):
                    i = g0 + t
                    dst = out_d.ap()[i * P * V : (i + 1) * P * V].rearrange(
                        "(p r) d -> p r d", r=V
                    )
                    nc.sync.dma_start(out=dst, in_=_bcast(y_out[:, t], V, 1))

    nc.compile()
    return nc


_NC_CACHE = {}


def _get_nc(n_tiles: int = N_TILES):
    if n_tiles not in _NC_CACHE:
        _NC_CACHE[n_tiles] = build_bass(n_tiles)
    return _NC_CACHE[n_tiles]


def _host_prep(Wq, Wk, Wv, Wo, bo):
    Wq = np.asarray(Wq, dtype=np.float32)
    Wk = np.asarray(Wk, dtype=np.float32)
    Wv = np.asarray(Wv, dtype=np.float32)
    Wo = np.asarray(Wo, dtype=np.float32)
    bo = np.asarray(bo, dtype=np.float32)
    scale = np.float32(1.0) / np.sqrt(np.float32(D))
    ma = (Wq.T @ Wk) * scale
    mq = Wq.T
    mamq = np.ascontiguousarray(
        np.concatenate([ma, mq], axis=1), dtype=np.float32
    )
    mb = np.ascontiguousarray(Wv.T @ Wo.T, dtype=np.float32)
    bo_b = np.ascontiguousarray(np.broadcast_to(bo, (P, D)), dtype=np.float32)
    ident = np.eye(P, dtype=np.float32)
    return mamq, mb, bo_b, ident


def make_in_maps(q, k, v, Wq, Wk, Wv, Wo, bo):
    q = np.asarray(q, dtype=np.float32)
    k = np.asarray(k, dtype=np.float32)
    v = np.asarray(v, dtype=np.float32)
    mamq, mb, bo_b, ident = _host_prep(Wq, Wk, Wv, Wo, bo)
    in_maps = []
    for c in range(N_CORES):
        sl = slice(c * NC_PTS, (c + 1) * NC_PTS)
        in_maps.append(
            {
                "q": np.ascontiguousarray(q[0, sl]),
                "k": np.ascontiguousarray(k[:, sl]),
                "v": np.ascontiguousarray(v[:, sl]),
                "mamq": mamq,
                "mb": mb,
                "bo_b": bo_b,
                "ident": ident,
            }
        )
    return in_maps


def kernel(q, k, v, Wq, Wk, Wv, Wo, bo):
    nc = _get_nc()
    in_maps = make_in_maps(q, k, v, Wq, Wk, Wv, Wo, bo)
    res = run_bass_kernel_spmd(nc, in_maps, core_ids=list(range(N_CORES)))
    return np.stack([r["out"] for r in res.results], axis=0)
